# revision 44
# baseline (speedup 1.0000x reference)
"""GQA attention kernel for Trainium2, data-parallel over batch on 8 NeuronCores.

Per-core problem (2 of 16 batches): X [1024tok, 1024] -> QKV proj -> RoPE ->
causal GQA attention (8 q heads, 4 kv heads, D=128) -> out proj [1024, 1024].

Layout strategy: everything stays in "feature-on-partition" transposed form,
and attention scores are computed TRANSPOSED (ST[tk,tq]) so that exp(ST) is
already the P.T the PV matmul needs -- no transposes of P at all. Matmul
operands are bf16 (fp32 PSUM accumulate).

Schedule (all per core):
  XT[hid,tok]   = host-pretransposed X                     (DRAM -> SBUF)
  QT[dq,tok]    = Wq.T @ XT   k-outer waves of 8 chains  + RoPE
  KT[dkv,tok]   = Wk.T @ XT   + RoPE
  V [tok,dkv]   = X @ Wv      (lhsT = XT, rhs = Wv)
  attention per (batch, kv-group) head PAIR, software-pipelined one pair deep:
    ST[tk, 2, tq] = KT_j.T @ QT_h  per 128-row tk block, causal col range,
                    both heads of the group into one 2-bank psum tile
    PT            = exp(ST)  one ACT op per (pair, j) via 3D AP
    mask          = one DVE mul per (pair, j) over both heads' diag blocks
                    (stride-0 broadcast mask operand)
    colsum[1,tq] += ones[128,1].T @ PT_j    (PE)
    OT[d,tq]     += V_j.T @ PT_j            (PE accumulate over j)
    norm per head, pipelined entirely off the PE:
      cs row -> scratch-DRAM roundtrip transpose -> WIDE reciprocal [128,4]
      -> roundtrip back -> DMA partition-broadcast [128,T] -> in-place
      SBUF multiply of the already-copied-out OT
  Out[tok,hid]  = OT.T @ Wo   -> bf16 store (host casts back to fp32)
RoPE scale 1/sqrt(D) is folded into the Q cos/sin host constants.
"""

import numpy as np
import ml_dtypes
from contextlib import ExitStack

import concourse.bass as bass
import concourse.tile as tile
from concourse import bacc, mybir
from concourse.bass_utils import run_bass_kernel_spmd

B, T, HID = 16, 512, 1024
NH, NKV, D = 8, 4, 128
THETA = 10000.0
NCORES = 8
BL = B // NCORES          # local batches per core
TOK = BL * T              # local tokens
P = 128
KT_HID = HID // P         # 8 contraction tiles over hidden
NTQ = T // P              # 4 tk/tq tiles per sequence
GROUPS = NH // NKV        # 2 q heads per kv head
NTOK_T = TOK // P         # 8 token tiles per core
FP32 = mybir.dt.float32
BF16 = mybir.dt.bfloat16
BF = ml_dtypes.bfloat16


def _host_consts():
    inv_freq = 1.0 / (THETA ** (np.arange(0, D, 2, dtype=np.float64) / D))
    freqs = np.outer(np.arange(T, dtype=np.float64), inv_freq)    # [T, 64]
    emb = np.concatenate([freqs, freqs], axis=-1)                 # [T, 128]
    cos = np.cos(emb).T                                           # [128, T]
    sin = np.sin(emb).T
    scale = 1.0 / np.sqrt(D)
    # rotate_half sign folded into sin: out = q*cos + qswap*sin_signed where
    # qswap is q with its partition halves swapped
    sin_signed = np.concatenate([-sin[:D // 2], sin[D // 2:]], axis=0)
    # transposed-S diagonal-block multiplicative mask: rows tk, cols tq;
    # valid iff tq >= tk
    mask_t = np.triu(np.ones((P, P), np.float32)).astype(BF)
    # cos|sin concatenated per projection: one DMA trigger each (HWDGE
    # triggers cost ~700ns of sequencer time apiece)
    return {
        "cs_q": np.concatenate(
            [(cos * scale), (sin_signed * scale)], axis=1).astype(BF),
        "cs_k": np.concatenate([cos, sin_signed], axis=1).astype(BF),
        "mask_t": mask_t,
    }


def _rope_wave(nc, outs, psums, cos_sb, sin_sb, tmp_pool):
    """out_i = q_i * cos + rotate_half(q_i) * sin for a whole 8-chain wave.

    ACT copies move each chain's psum into one contiguous [P, 8, T] buffer
    (single slow PSUM read each), then the rotate_half partition-half swap
    is TWO wave-sized DMAs (compute engines cannot shift partitions; and
    per-chain swaps would cost 16 DMA triggers at ~700ns of sequencer time
    each). The sign of rotate_half is folded into the host sin constant.
    The arithmetic runs in the DVE's fast bf16-SBUF mode.
    """
    H = D // 2
    n = len(outs)
    qraw = tmp_pool.tile([P, n, T], BF16, tag="rope_raw", bufs=1)
    qswap = tmp_pool.tile([P, n, T], BF16, tag="rope_swap", bufs=1)
    for i in range(n):
        nc.scalar.copy(qraw[:, i, :], psums[i])
    nc.scalar.dma_start(out=qswap[0:H], in_=qraw[H:P])
    nc.scalar.dma_start(out=qswap[H:P], in_=qraw[0:H])
    for i in range(n):
        tmp = tmp_pool.tile([P, T], BF16, tag="rope_tmp")
        nc.gpsimd.tensor_mul(tmp, qswap[:, i, :], sin_sb)
        nc.vector.tensor_mul(outs[i], qraw[:, i, :], cos_sb)
        nc.vector.tensor_add(outs[i], outs[i], tmp)


def _build(nc):
    # hidden arrives pre-transposed from the host: [HID, TOK]
    hid_t = nc.dram_tensor("hidden_t", [HID, TOK], BF16,
                           kind="ExternalInput").ap()
    wq = nc.dram_tensor("Wq", [HID, NH * D], BF16, kind="ExternalInput").ap()
    wk = nc.dram_tensor("Wk", [HID, NKV * D], BF16, kind="ExternalInput").ap()
    wv = nc.dram_tensor("Wv", [HID, NKV * D], BF16, kind="ExternalInput").ap()
    wo = nc.dram_tensor("Wo", [NH * D, HID], BF16, kind="ExternalInput").ap()
    cs_q = nc.dram_tensor("cs_q", [P, 2 * T], BF16, kind="ExternalInput").ap()
    cs_k = nc.dram_tensor("cs_k", [P, 2 * T], BF16, kind="ExternalInput").ap()
    mask_t = nc.dram_tensor("mask_t", [P, P], BF16, kind="ExternalInput").ap()
    out = nc.dram_tensor("out", [TOK, HID], BF16, kind="ExternalOutput").ap()

    with tile.TileContext(nc) as tc, ExitStack() as ctx:
        # ---- pools with cross-phase lifetimes ----
        consts = ctx.enter_context(tc.tile_pool(name="consts", bufs=1))

        csq_sb = consts.tile([P, 2 * T], BF16, tag="cq")
        csk_sb = consts.tile([P, 2 * T], BF16, tag="ck")
        cosq_sb, sinq_sb = csq_sb[:, 0:T], csq_sb[:, T:2 * T]
        cosk_sb, sink_sb = csk_sb[:, 0:T], csk_sb[:, T:2 * T]
        maskt_sb = consts.tile([P, P], BF16, tag="maskt")
        ones_bf = consts.tile([P, P], BF16, tag="ones")
        warm_rhs = consts.tile([P, 256], BF16, tag="warm")
        # memsets on the (otherwise idle at startup) gpsimd engine so the
        # PE warmup can begin as early as possible
        nc.gpsimd.memset(ones_bf, 1.0)
        nc.gpsimd.memset(warm_rhs, 0.0)

        qkvpool = ctx.enter_context(tc.tile_pool(name="qkv", bufs=1))
        # per-BATCH q/k tiles: dependency tracking is tile-granular, so a
        # single [.., TOK] tile would make batch 0's first ST matmul wait
        # for batch 1's (much later) RoPE writes
        qts = [qkvpool.tile([P, NH, T], BF16, tag=f"qt{c}", name=f"qt{c}")
               for c in range(BL)]                             # [d, h, t]
        kts = [qkvpool.tile([P, NKV, T], BF16, tag=f"kt{c}", name=f"kt{c}")
               for c in range(BL)]                             # [d, g, t]
        v_sb = qkvpool.tile([P, NTOK_T, NKV * D], BF16, tag="v")  # [tok,tt,dkv]

        # ---- phase A+B: loads + QKV projections (k-outer waves) ----
        with ExitStack() as phase1:
            wpool = phase1.enter_context(tc.tile_pool(name="wpool", bufs=1))
            xtp = phase1.enter_context(tc.tile_pool(name="xtp", bufs=1))
            ropet = phase1.enter_context(tc.tile_pool(name="ropet", bufs=6))
            psB = phase1.enter_context(
                tc.tile_pool(name="psB", bufs=8, space=bass.MemorySpace.PSUM))

            wq_sb = wpool.tile([P, KT_HID, NH * D], BF16, tag="wq")
            wk_sb = wpool.tile([P, KT_HID, NKV * D], BF16, tag="wk")
            wv_sb = wpool.tile([P, KT_HID, NKV * D], BF16, tag="wv")
            xt_sb = xtp.tile([P, KT_HID, TOK], BF16, tag="xt")  # [hid, k, tok]
            wq_r = wq.rearrange("(k p) n -> p k n", p=P)
            wk_r = wk.rearrange("(k p) n -> p k n", p=P)
            wv_r = wv.rearrange("(k p) n -> p k n", p=P)
            hid_r = hid_t.rearrange("(k p) t -> p k t", p=P)
            # load order follows consumption order: the Q projection streams
            # k-chunk by k-chunk, so (xt[k], wq[k]) pairs go first, split
            # across the two HARDWARE DGE rings (sync + scalar; the gpsimd
            # ring is software DGE executed on the Q7 cores -- never use it
            # for bulk). RoPE consts next; then wk/wv/wo.
            # X ships in batch-halves: wave Q(c=0) needs only xt[:, k, 0:T],
            # so its working set streams ahead of PE consumption. Few, BIG
            # triggers: each dma_start not only costs ~700ns of sequencer
            # time, it can BLOCK the issuing engine until ring slots free.
            # So the scalar ring carries ONLY wq (done by the time the ACT
            # engine must run RoPE copies); everything else rides sync.
            KH = KT_HID // 2
            nc.sync.dma_start(out=xt_sb[:, 0:KH, 0:T],
                              in_=hid_r[:, 0:KH, 0:T])
            nc.scalar.dma_start(out=wq_sb[:, 0:2, :], in_=wq_r[:, 0:2, :])
            nc.scalar.dma_start(out=wq_sb[:, 2:4, :], in_=wq_r[:, 2:4, :])
            nc.sync.dma_start(out=xt_sb[:, KH:KT_HID, 0:T],
                              in_=hid_r[:, KH:KT_HID, 0:T])
            nc.scalar.dma_start(out=wq_sb[:, 4:6, :], in_=wq_r[:, 4:6, :])
            nc.scalar.dma_start(out=wq_sb[:, 6:8, :], in_=wq_r[:, 6:8, :])
            nc.sync.dma_start(out=xt_sb[:, :, T:TOK], in_=hid_r[:, :, T:TOK])
            nc.sync.dma_start(out=csq_sb, in_=cs_q)
            nc.sync.dma_start(out=csk_sb, in_=cs_k)
            nc.sync.dma_start(out=maskt_sb, in_=mask_t)
            nc.sync.dma_start(out=wk_sb, in_=wk_r)
            nc.sync.dma_start(out=wv_sb, in_=wv_r)

            # PE warmup: ~3.5us of dependency-light matmuls ahead of the
            # first projection so the HAM clock-gate releases (1.2 ->
            # 2.4 GHz) while the input DMAs are still in flight
            wps = psB.tile([P, T], FP32, tag="projps")
            for _ in range(20):
                nc.tensor.matmul(wps[:, 0:256], ones_bf, warm_rhs,
                                 start=True, stop=True, skip_group_check=True)

            def q_wave(c):
                # k-outer: the first matmuls need only (xt[0], wq[0]), so
                # the PE starts when the first 512KB lands, not after the
                # whole load; DMA delivery and PE consumption are balanced.
                pss = [psB.tile([P, T], FP32, tag="projps",
                                name=f"psq{c}_{i}") for i in range(NH)]
                for k in range(KT_HID):
                    for h in range(NH):
                        nc.tensor.matmul(
                            pss[h],
                            wq_sb[:, k, h * P:(h + 1) * P],
                            xt_sb[:, k, c * T:(c + 1) * T],
                            start=(k == 0), stop=(k == KT_HID - 1))
                _rope_wave(nc, [qts[c][:, h, :] for h in range(NH)],
                           pss, cosq_sb, sinq_sb, ropet)

            def k_wave():
                chains = [(g, cc) for g in range(NKV) for cc in range(BL)]
                pss = [psB.tile([P, T], FP32, tag="projps", name=f"psk{i}")
                       for i in range(len(chains))]
                for k in range(KT_HID):
                    for i, (g, cc) in enumerate(chains):
                        nc.tensor.matmul(
                            pss[i],
                            wk_sb[:, k, g * P:(g + 1) * P],
                            xt_sb[:, k, cc * T:(cc + 1) * T],
                            start=(k == 0), stop=(k == KT_HID - 1))
                _rope_wave(nc, [kts[cc][:, g, :] for g, cc in chains],
                           pss, cosk_sb, sink_sb, ropet)

            def v_wave():
                pss = [psB.tile([P, T], FP32, tag="projps", name=f"psv{i}")
                       for i in range(NTOK_T)]
                for k in range(KT_HID):
                    for tt in range(NTOK_T):
                        nc.tensor.matmul(
                            pss[tt][:, :NKV * D],
                            xt_sb[:, k, tt * P:(tt + 1) * P],
                            wv_sb[:, k, :],
                            start=(k == 0), stop=(k == KT_HID - 1))
                for tt in range(NTOK_T):
                    # alternate copy engines so the drain is not ACT-serial
                    if tt % 2 == 0:
                        nc.scalar.copy(v_sb[:, tt, :], pss[tt][:, :NKV * D])
                    else:
                        nc.vector.tensor_copy(v_sb[:, tt, :],
                                              pss[tt][:, :NKV * D])

            # K early: its wave-swap DMA is 2nd in the scalar ring so kt is
            # RoPE'd well before the first ST matmul. V last: the attention
            # phase's first psum allocations alias the last wave's banks,
            # and V's copies drain fastest (a RoPE tail would stall the
            # first ST matmuls ~7us).
            q_wave(0)
            k_wave()
            q_wave(1)
            v_wave()

        # ---- phase C: attention, head pairs, one-pair software pipeline ----
        otpool = ctx.enter_context(tc.tile_pool(name="otpool", bufs=1))
        ot_sb = otpool.tile([P, NH, TOK], BF16, tag="ot")      # [d, h, tok]
        wopool = ctx.enter_context(tc.tile_pool(name="wopool", bufs=1))
        wo_sb = wopool.tile([P, KT_HID, HID], BF16, tag="wo")
        # normalization state lives past phase C (batch 1's normalization
        # overlaps batch 0's output projection)
        normp = ctx.enter_context(tc.tile_pool(name="normp", bufs=8))
        sums = [normp.tile([NH, T], FP32, tag=f"sums{b}", name=f"sums{b}")
                for b in range(BL)]
        psR = ctx.enter_context(
            tc.tile_pool(name="psR", bufs=2, space=bass.MemorySpace.PSUM))

        with ExitStack() as phase2:
            ptpool = phase2.enter_context(tc.tile_pool(name="ptpool", bufs=2))
            # PSUM budget is exactly 8 banks: ST pair tiles 2x2 (per-j
            # tiles, double-buffered, so ST(j+1) never waits for exp(j) to
            # drain an overlapping region) + o_ps 1 + cs 1 (the cs/OT/copy
            # interleave within a pair covers the single-buffer WARs) +
            # rank-1 broadcast 2 (outer pool).
            psS = phase2.enter_context(
                tc.tile_pool(name="psS", bufs=2, space=bass.MemorySpace.PSUM))
            psO = phase2.enter_context(
                tc.tile_pool(name="psO", bufs=1, space=bass.MemorySpace.PSUM))
            psC = phase2.enter_context(
                tc.tile_pool(name="psC", bufs=1, space=bass.MemorySpace.PSUM))

            mask_b = maskt_sb[:, None, :].to_broadcast((P, GROUPS, P))

            def emit_st(b, g):
                """ST matmuls + exp + mask for one head pair; returns pt."""
                pt = ptpool.tile([P, GROUPS, NTQ, T], BF16, tag="pt")
                for j in range(NTQ):
                    lo = j * P
                    st = psS.tile([P, GROUPS, T], FP32, tag="sps")
                    for hh in range(GROUPS):
                        h = GROUPS * g + hh
                        nc.tensor.matmul(
                            st[:, hh, lo:T],
                            kts[b][:, g, lo: lo + P],
                            qts[b][:, h, lo:T],
                            start=True, stop=True)
                    # one exp per (pair, j): 3D AP spanning both psum banks
                    # (no row-max: logits are O(1) by construction)
                    nc.scalar.activation(
                        out=pt[:, :, j, lo:T], in_=st[:, :, lo:T],
                        func=mybir.ActivationFunctionType.Exp,
                        bias=0.0, scale=1.0)
                    # causal mask on the diagonal block, both heads in one
                    # op (mask operand broadcast along the head axis), on
                    # the otherwise-idle gpsimd
                    nc.gpsimd.tensor_mul(
                        pt[:, :, j, lo:lo + P], pt[:, :, j, lo:lo + P],
                        mask_b)
                return pt

            def emit_csot_mm(b, g, pt):
                """colsum + OT accumulation; denominator stashed for the
                batched normalization."""
                for hh in range(GROUPS):
                    h = GROUPS * g + hh
                    o_ps = psO.tile([P, T], FP32, tag="ops")
                    cs_ps = psC.tile([1, T], FP32, tag="cps")
                    # colsum matmuls first so the denominator stash launches
                    # before the OT matmuls run
                    for j in range(NTQ):
                        lo = j * P
                        nc.tensor.matmul(
                            cs_ps[:, lo:T] if j else cs_ps[:, :],
                            ones_bf[:, 0:1],
                            pt[:, hh, j, lo:T],
                            start=(j == 0), stop=(j == NTQ - 1),
                            skip_group_check=True)
                    # copy the [1,T] row out of PSUM (split across ACT/DVE)
                    # and DMA-stash it into partition h of sums[b] so the
                    # whole batch's reciprocal runs 8-lane-wide later
                    # h0's copy on DVE (first DVE op of the pair, runs
                    # early), h1's on ACT: the next pair's first cs matmul
                    # WARs on h1's copy through the single cs bank, and ACT
                    # reaches it sooner than the backlogged DVE
                    csrow = normp.tile([1, T], FP32, tag="csrow")
                    if hh == 0:
                        nc.vector.tensor_copy(csrow, cs_ps)
                    else:
                        nc.scalar.copy(csrow, cs_ps)
                    nc.sync.dma_start(out=sums[b][h:h + 1, :], in_=csrow)
                    for j in range(NTQ):
                        lo = j * P
                        nc.tensor.matmul(
                            o_ps[:, lo:T] if j else o_ps[:, :],
                            v_sb[:, b * NTQ + j, g * D:(g + 1) * D],
                            pt[:, hh, j, lo:T],
                            start=(j == 0), stop=(j == NTQ - 1),
                            skip_group_check=True)
                    # unnormalized OT out of PSUM immediately (frees banks;
                    # the normalization multiply lands later, in-place)
                    nc.vector.tensor_copy(
                        ot_sb[:, h, b * T:(b + 1) * T], o_ps)

            def emit_batch_recip(b):
                """one 8-lane-wide reciprocal for the whole batch, then
                extract each head's row back to partition 0 for the rank-1
                broadcast. Runs while the next batch (or the output
                projection) keeps the PE busy."""
                rinv_f = normp.tile([NH, T], FP32, tag="rinvf",
                                    name=f"rinvf{b}")
                nc.vector.reciprocal_approx_fast(out=rinv_f, in_=sums[b])
                rinv_bf = normp.tile([NH, T], BF16, tag="rinvbf",
                                     name=f"rinvbf{b}")
                nc.vector.tensor_copy(rinv_bf, rinv_f)
                rrows = []
                for h in range(NH):
                    rrow = normp.tile([1, T], BF16, tag="rrow",
                                      name=f"rrow{b}_{h}")
                    nc.sync.dma_start(out=rrow, in_=rinv_bf[h:h + 1, :])
                    rrows.append(rrow)
                return rrows

            def emit_norm_heads(b, rrows, heads):
                """rank-1 broadcast + in-place multiply for a few heads;
                chunks are interspersed between matmul blocks so the PE
                never waits on the (cheap but latent) reciprocal chain."""
                for h in heads:
                    rb_ps = psR.tile([P, T], FP32, tag="rbps")
                    nc.tensor.matmul(rb_ps, ones_bf[0:1, :], rrows[h],
                                     start=True, stop=True,
                                     skip_group_check=True)
                    nc.vector.tensor_mul(
                        ot_sb[:, h, b * T:(b + 1) * T],
                        ot_sb[:, h, b * T:(b + 1) * T], rb_ps)

            # software-pipelined schedule: csot lags ST by one pair; batch
            # 0's normalization hides under batch 1's attention; batch 1's
            # hides under batch 0's output projection (emitted in phase D)
            pts = {}
            pairs = [(b, g) for b in range(BL) for g in range(NKV)]
            for b, g in pairs:
                pts[(b, g)] = None
            # wo's 2MB load trigger goes on the sync ring HERE: behind the
            # phase-1 loads but ahead of the norm stashes, landing long
            # before phase D needs it without blocking anything hot
            nc.sync.dma_start(out=wo_sb,
                              in_=wo.rearrange("(k p) n -> p k n", p=P))
            pts[(0, 0)] = emit_st(0, 0)
            for i in range(1, 4):
                pts[(0, i)] = emit_st(0, i)
                emit_csot_mm(0, i - 1, pts[(0, i - 1)])
            pts[(1, 0)] = emit_st(1, 0)
            emit_csot_mm(0, 3, pts[(0, 3)])
            pts[(1, 1)] = emit_st(1, 1)
            emit_csot_mm(1, 0, pts[(1, 0)])
            rrows0 = emit_batch_recip(0)
            emit_norm_heads(0, rrows0, [0, 1])
            pts[(1, 2)] = emit_st(1, 2)
            emit_csot_mm(1, 1, pts[(1, 1)])
            emit_norm_heads(0, rrows0, [2, 3])
            pts[(1, 3)] = emit_st(1, 3)
            emit_csot_mm(1, 2, pts[(1, 2)])
            emit_norm_heads(0, rrows0, [4, 5])
            emit_csot_mm(1, 3, pts[(1, 3)])
            emit_norm_heads(0, rrows0, [6, 7])
            rrows1 = emit_batch_recip(1)

        # ---- phase D: output projection (b0 tiles overlap b1's norm) ----
        with ExitStack() as phase3:
            opool = phase3.enter_context(tc.tile_pool(name="opool", bufs=3))
            psD = phase3.enter_context(
                tc.tile_pool(name="psD", bufs=3, space=bass.MemorySpace.PSUM))
            NCH = HID // T  # 2 chunks of 512

            def emit_out_tile(tt):
                o_tile = opool.tile([P, HID], BF16, tag="o")
                # interleave both output chunks k-major: consecutive matmul
                # pairs share the stationary operand OT[:,k,tt-block]
                ps0 = psD.tile([P, T], FP32, tag="dps0")
                ps1 = psD.tile([P, T], FP32, tag="dps1")
                pss = [ps0, ps1]
                for k in range(KT_HID):
                    for cchunk in range(NCH):
                        nc.tensor.matmul(
                            pss[cchunk],
                            ot_sb[:, k, tt * P:(tt + 1) * P],
                            wo_sb[:, k, cchunk * T:(cchunk + 1) * T],
                            start=(k == 0), stop=(k == KT_HID - 1))
                # alternate engines so the copies run in parallel; the last
                # tile is pure tail, so chunk it across engines and rings
                if tt < NTOK_T - 1:
                    nc.vector.tensor_copy(o_tile[:, 0:T], pss[0])
                    nc.scalar.copy(o_tile[:, T:HID], pss[1])
                    eng = nc.sync if tt % 2 == 0 else nc.scalar
                    eng.dma_start(out=out[tt * P:(tt + 1) * P, :], in_=o_tile)
                else:
                    HT = T // 2
                    nc.vector.tensor_copy(o_tile[:, 0:HT], pss[0][:, 0:HT])
                    nc.scalar.copy(o_tile[:, HT:T], pss[0][:, HT:T])
                    nc.vector.tensor_copy(o_tile[:, T:T + HT],
                                          pss[1][:, 0:HT])
                    nc.scalar.copy(o_tile[:, T + HT:HID], pss[1][:, HT:T])
                    nc.sync.dma_start(out=out[tt * P:(tt + 1) * P, 0:T],
                                      in_=o_tile[:, 0:T])
                    nc.scalar.dma_start(out=out[tt * P:(tt + 1) * P, T:HID],
                                        in_=o_tile[:, T:HID])

            emit_out_tile(0)
            emit_norm_heads(1, rrows1, [0, 1])
            emit_out_tile(1)
            emit_norm_heads(1, rrows1, [2, 3])
            emit_out_tile(2)
            emit_norm_heads(1, rrows1, [4, 5])
            emit_out_tile(3)
            emit_norm_heads(1, rrows1, [6, 7])
            for tt in range(4, NTOK_T):
                emit_out_tile(tt)


_COMPILED = None


def _get_compiled():
    global _COMPILED
    if _COMPILED is None:
        nc = bacc.Bacc("TRN2", target_bir_lowering=False, debug=False)
        _build(nc)
        nc.compile()
        _COMPILED = nc
    return _COMPILED


def kernel(hidden_states, Wq, Wk, Wv, Wo, _trace=False, _trace_kwargs=None):
    hs = np.asarray(hidden_states, dtype=np.float32).astype(BF)
    wq = np.ascontiguousarray(np.asarray(Wq, dtype=np.float32).astype(BF))
    wk = np.ascontiguousarray(np.asarray(Wk, dtype=np.float32).astype(BF))
    wv = np.ascontiguousarray(np.asarray(Wv, dtype=np.float32).astype(BF))
    wo = np.ascontiguousarray(np.asarray(Wo, dtype=np.float32).astype(BF))
    consts = _host_consts()
    nc = _get_compiled()
    in_maps = []
    for c in range(NCORES):
        # ship X pre-transposed ([HID, TOK]) so the kernel's lhs/rhs layouts
        # need no on-chip transpose of X at all
        shard_t = np.ascontiguousarray(
            hs[BL * c: BL * (c + 1)].reshape(TOK, HID).T)
        in_maps.append({"hidden_t": shard_t, "Wq": wq, "Wk": wk, "Wv": wv,
                        "Wo": wo, **consts})
    res = run_bass_kernel_spmd(
        nc, in_maps, list(range(NCORES)), trace=_trace,
        **(_trace_kwargs or {}))
    outs = [r["out"].astype(np.float32).reshape(BL, T, HID)
            for r in res.results]
    full = np.concatenate(outs, axis=0)
    if _trace:
        return full, res
    return full


# revision 45
# speedup vs baseline: 1.2059x; 1.2059x over previous
"""GQA attention kernel for Trainium2, data-parallel over batch on 8 NeuronCores.

Per-core problem (2 of 16 batches): X [1024tok, 1024] -> QKV proj -> RoPE ->
causal GQA attention (8 q heads, 4 kv heads, D=128) -> out proj [1024, 1024].

Layout strategy: everything stays in "feature-on-partition" transposed form,
and attention scores are computed TRANSPOSED (ST[tk,tq]) so that exp(ST) is
already the P.T the PV matmul needs -- no transposes of P at all. Matmul
operands are bf16 (fp32 PSUM accumulate).

Schedule (all per core):
  XT[hid,tok]   = host-pretransposed X                     (DRAM -> SBUF)
  QT[dq,tok]    = Wq.T @ XT   k-outer waves of 8 chains  + RoPE
  KT[dkv,tok]   = Wk.T @ XT   + RoPE
  V [tok,dkv]   = X @ Wv      (lhsT = XT, rhs = Wv)
  attention per (batch, kv-group) head PAIR, software-pipelined one pair deep:
    ST[tk, 2, tq] = KT_j.T @ QT_h  per 128-row tk block, causal col range,
                    both heads of the group into one 2-bank psum tile
    PT            = exp(ST)  one ACT op per (pair, j) via 3D AP
    mask          = one DVE mul per (pair, j) over both heads' diag blocks
                    (stride-0 broadcast mask operand)
    colsum[1,tq] += ones[128,1].T @ PT_j    (PE)
    OT[d,tq]     += V_j.T @ PT_j            (PE accumulate over j)
    norm per head, pipelined entirely off the PE:
      cs row -> scratch-DRAM roundtrip transpose -> WIDE reciprocal [128,4]
      -> roundtrip back -> DMA partition-broadcast [128,T] -> in-place
      SBUF multiply of the already-copied-out OT
  Out[tok,hid]  = OT.T @ Wo   -> bf16 store (host casts back to fp32)
RoPE scale 1/sqrt(D) is folded into the Q cos/sin host constants.
"""

import numpy as np
import ml_dtypes
from contextlib import ExitStack

import concourse.bass as bass
import concourse.tile as tile
from concourse import bacc, mybir
from concourse.bass_utils import run_bass_kernel_spmd

B, T, HID = 16, 512, 1024
NH, NKV, D = 8, 4, 128
THETA = 10000.0
NCORES = 8
BL = B // NCORES          # local batches per core
TOK = BL * T              # local tokens
P = 128
KT_HID = HID // P         # 8 contraction tiles over hidden
NTQ = T // P              # 4 tk/tq tiles per sequence
GROUPS = NH // NKV        # 2 q heads per kv head
NTOK_T = TOK // P         # 8 token tiles per core
FP32 = mybir.dt.float32
BF16 = mybir.dt.bfloat16
BF = ml_dtypes.bfloat16


def _host_consts():
    inv_freq = 1.0 / (THETA ** (np.arange(0, D, 2, dtype=np.float64) / D))
    freqs = np.outer(np.arange(T, dtype=np.float64), inv_freq)    # [T, 64]
    emb = np.concatenate([freqs, freqs], axis=-1)                 # [T, 128]
    cos = np.cos(emb).T                                           # [128, T]
    sin = np.sin(emb).T
    scale = 1.0 / np.sqrt(D)
    # rotate_half sign folded into sin: out = q*cos + qswap*sin_signed where
    # qswap is q with its partition halves swapped
    sin_signed = np.concatenate([-sin[:D // 2], sin[D // 2:]], axis=0)
    # transposed-S diagonal-block multiplicative mask: rows tk, cols tq;
    # valid iff tq >= tk
    mask_t = np.triu(np.ones((P, P), np.float32)).astype(BF)
    # cos|sin concatenated per projection: one DMA trigger each (HWDGE
    # triggers cost ~700ns of sequencer time apiece)
    return {
        "cs_q": np.concatenate(
            [(cos * scale), (sin_signed * scale)], axis=1).astype(BF),
        "cs_k": np.concatenate([cos, sin_signed], axis=1).astype(BF),
        "mask_t": mask_t,
    }


def _rope_wave(nc, outs, psums, cos_sb, sin_sb, tmp_pool):
    """out_i = q_i * cos + rotate_half(q_i) * sin for a whole 8-chain wave.

    ACT copies move each chain's psum into one contiguous [P, 8, T] buffer
    (single slow PSUM read each), then the rotate_half partition-half swap
    is TWO wave-sized DMAs (compute engines cannot shift partitions; and
    per-chain swaps would cost 16 DMA triggers at ~700ns of sequencer time
    each). The sign of rotate_half is folded into the host sin constant.
    The arithmetic runs in the DVE's fast bf16-SBUF mode.
    """
    H = D // 2
    n = len(outs)
    qraw = tmp_pool.tile([P, n, T], BF16, tag="rope_raw", bufs=1)
    qswap = tmp_pool.tile([P, n, T], BF16, tag="rope_swap", bufs=1)
    for i in range(n):
        nc.scalar.copy(qraw[:, i, :], psums[i])
    nc.scalar.dma_start(out=qswap[0:H], in_=qraw[H:P])
    nc.scalar.dma_start(out=qswap[H:P], in_=qraw[0:H])
    for i in range(n):
        tmp = tmp_pool.tile([P, T], BF16, tag="rope_tmp")
        nc.gpsimd.tensor_mul(tmp, qswap[:, i, :], sin_sb)
        nc.vector.tensor_mul(outs[i], qraw[:, i, :], cos_sb)
        nc.vector.tensor_add(outs[i], outs[i], tmp)


def _build(nc):
    # hidden arrives pre-transposed from the host: [HID, TOK]
    hid_t = nc.dram_tensor("hidden_t", [HID, TOK], BF16,
                           kind="ExternalInput").ap()
    wq = nc.dram_tensor("Wq", [HID, NH * D], BF16, kind="ExternalInput").ap()
    wk = nc.dram_tensor("Wk", [HID, NKV * D], BF16, kind="ExternalInput").ap()
    wv = nc.dram_tensor("Wv", [HID, NKV * D], BF16, kind="ExternalInput").ap()
    wo = nc.dram_tensor("Wo", [NH * D, HID], BF16, kind="ExternalInput").ap()
    cs_q = nc.dram_tensor("cs_q", [P, 2 * T], BF16, kind="ExternalInput").ap()
    cs_k = nc.dram_tensor("cs_k", [P, 2 * T], BF16, kind="ExternalInput").ap()
    mask_t = nc.dram_tensor("mask_t", [P, P], BF16, kind="ExternalInput").ap()
    out = nc.dram_tensor("out", [TOK, HID], BF16, kind="ExternalOutput").ap()

    with tile.TileContext(nc) as tc, ExitStack() as ctx:
        # ---- pools with cross-phase lifetimes ----
        consts = ctx.enter_context(tc.tile_pool(name="consts", bufs=1))

        csq_sb = consts.tile([P, 2 * T], BF16, tag="cq")
        csk_sb = consts.tile([P, 2 * T], BF16, tag="ck")
        cosq_sb, sinq_sb = csq_sb[:, 0:T], csq_sb[:, T:2 * T]
        cosk_sb, sink_sb = csk_sb[:, 0:T], csk_sb[:, T:2 * T]
        maskt_sb = consts.tile([P, P], BF16, tag="maskt")
        ones_bf = consts.tile([P, P], BF16, tag="ones")
        warm_rhs = consts.tile([P, 256], BF16, tag="warm")
        # memsets on the (otherwise idle at startup) gpsimd engine so the
        # PE warmup can begin as early as possible
        nc.gpsimd.memset(ones_bf, 1.0)
        nc.gpsimd.memset(warm_rhs, 0.0)

        qkvpool = ctx.enter_context(tc.tile_pool(name="qkv", bufs=1))
        # per-BATCH q/k tiles: dependency tracking is tile-granular, so a
        # single [.., TOK] tile would make batch 0's first ST matmul wait
        # for batch 1's (much later) RoPE writes
        qts = [qkvpool.tile([P, NH, T], BF16, tag=f"qt{c}", name=f"qt{c}")
               for c in range(BL)]                             # [d, h, t]
        kts = [qkvpool.tile([P, NKV, T], BF16, tag=f"kt{c}", name=f"kt{c}")
               for c in range(BL)]                             # [d, g, t]
        v_sb = qkvpool.tile([P, NTOK_T, NKV * D], BF16, tag="v")  # [tok,tt,dkv]

        # ---- phase A+B: loads + QKV projections (k-outer waves) ----
        with ExitStack() as phase1:
            wpool = phase1.enter_context(tc.tile_pool(name="wpool", bufs=1))
            xtp = phase1.enter_context(tc.tile_pool(name="xtp", bufs=1))
            ropet = phase1.enter_context(tc.tile_pool(name="ropet", bufs=6))
            psB = phase1.enter_context(
                tc.tile_pool(name="psB", bufs=8, space=bass.MemorySpace.PSUM))

            wq_sb = wpool.tile([P, KT_HID, NH * D], BF16, tag="wq")
            wk_sb = wpool.tile([P, KT_HID, NKV * D], BF16, tag="wk")
            wv_sb = wpool.tile([P, KT_HID, NKV * D], BF16, tag="wv")
            xt_sb = xtp.tile([P, KT_HID, TOK], BF16, tag="xt")  # [hid, k, tok]
            wq_r = wq.rearrange("(k p) n -> p k n", p=P)
            wk_r = wk.rearrange("(k p) n -> p k n", p=P)
            wv_r = wv.rearrange("(k p) n -> p k n", p=P)
            hid_r = hid_t.rearrange("(k p) t -> p k t", p=P)
            # load order follows consumption order: the Q projection streams
            # k-chunk by k-chunk, so (xt[k], wq[k]) pairs go first, split
            # across the two HARDWARE DGE rings (sync + scalar; the gpsimd
            # ring is software DGE executed on the Q7 cores -- never use it
            # for bulk). RoPE consts next; then wk/wv/wo.
            # X ships in batch-halves: wave Q(c=0) needs only xt[:, k, 0:T],
            # so its working set streams ahead of PE consumption. Few, BIG
            # triggers: each dma_start not only costs ~700ns of sequencer
            # time, it can BLOCK the issuing engine until ring slots free.
            # So the scalar ring carries ONLY wq (done by the time the ACT
            # engine must run RoPE copies); everything else rides sync.
            KH = KT_HID // 2
            nc.sync.dma_start(out=xt_sb[:, 0:KH, 0:T],
                              in_=hid_r[:, 0:KH, 0:T])
            nc.scalar.dma_start(out=wq_sb[:, 0:2, :], in_=wq_r[:, 0:2, :])
            nc.scalar.dma_start(out=wq_sb[:, 2:4, :], in_=wq_r[:, 2:4, :])
            nc.sync.dma_start(out=xt_sb[:, KH:KT_HID, 0:T],
                              in_=hid_r[:, KH:KT_HID, 0:T])
            nc.scalar.dma_start(out=wq_sb[:, 4:6, :], in_=wq_r[:, 4:6, :])
            nc.scalar.dma_start(out=wq_sb[:, 6:8, :], in_=wq_r[:, 6:8, :])
            nc.sync.dma_start(out=xt_sb[:, :, T:TOK], in_=hid_r[:, :, T:TOK])
            nc.sync.dma_start(out=csq_sb, in_=cs_q)
            nc.sync.dma_start(out=csk_sb, in_=cs_k)
            nc.sync.dma_start(out=maskt_sb, in_=mask_t)
            nc.sync.dma_start(out=wk_sb, in_=wk_r)
            nc.sync.dma_start(out=wv_sb, in_=wv_r)

            # PE warmup: ~3.5us of dependency-light matmuls ahead of the
            # first projection so the HAM clock-gate releases (1.2 ->
            # 2.4 GHz) while the input DMAs are still in flight
            wps = psB.tile([P, T], FP32, tag="projps")
            for _ in range(20):
                nc.tensor.matmul(wps[:, 0:256], ones_bf, warm_rhs,
                                 start=True, stop=True, skip_group_check=True)

            def q_wave(c):
                # k-outer: the first matmuls need only (xt[0], wq[0]), so
                # the PE starts when the first 512KB lands, not after the
                # whole load; DMA delivery and PE consumption are balanced.
                pss = [psB.tile([P, T], FP32, tag="projps",
                                name=f"psq{c}_{i}") for i in range(NH)]
                for k in range(KT_HID):
                    for h in range(NH):
                        nc.tensor.matmul(
                            pss[h],
                            wq_sb[:, k, h * P:(h + 1) * P],
                            xt_sb[:, k, c * T:(c + 1) * T],
                            start=(k == 0), stop=(k == KT_HID - 1))
                _rope_wave(nc, [qts[c][:, h, :] for h in range(NH)],
                           pss, cosq_sb, sinq_sb, ropet)

            def k_wave():
                chains = [(g, cc) for g in range(NKV) for cc in range(BL)]
                pss = [psB.tile([P, T], FP32, tag="projps", name=f"psk{i}")
                       for i in range(len(chains))]
                for k in range(KT_HID):
                    for i, (g, cc) in enumerate(chains):
                        nc.tensor.matmul(
                            pss[i],
                            wk_sb[:, k, g * P:(g + 1) * P],
                            xt_sb[:, k, cc * T:(cc + 1) * T],
                            start=(k == 0), stop=(k == KT_HID - 1))
                _rope_wave(nc, [kts[cc][:, g, :] for g, cc in chains],
                           pss, cosk_sb, sink_sb, ropet)

            def v_wave():
                pss = [psB.tile([P, T], FP32, tag="projps", name=f"psv{i}")
                       for i in range(NTOK_T)]
                for k in range(KT_HID):
                    for tt in range(NTOK_T):
                        nc.tensor.matmul(
                            pss[tt][:, :NKV * D],
                            xt_sb[:, k, tt * P:(tt + 1) * P],
                            wv_sb[:, k, :],
                            start=(k == 0), stop=(k == KT_HID - 1))
                for tt in range(NTOK_T):
                    # all on ACT: a DVE copy here would queue behind Q1's
                    # RoPE arithmetic (itself gated on its late wave-swap
                    # DMA) and stall the attention phase's first psum reuse
                    nc.scalar.copy(v_sb[:, tt, :], pss[tt][:, :NKV * D])

            # K early: its wave-swap DMA is 2nd in the scalar ring so kt is
            # RoPE'd well before the first ST matmul. V last: the attention
            # phase's first psum allocations alias the last wave's banks,
            # and V's copies drain fastest (a RoPE tail would stall the
            # first ST matmuls ~7us).
            q_wave(0)
            k_wave()
            q_wave(1)
            v_wave()

        # ---- phase C: attention, head pairs, one-pair software pipeline ----
        otpool = ctx.enter_context(tc.tile_pool(name="otpool", bufs=1))
        ot_sb = otpool.tile([P, NH, TOK], BF16, tag="ot")      # [d, h, tok]
        wopool = ctx.enter_context(tc.tile_pool(name="wopool", bufs=1))
        wo_sb = wopool.tile([P, KT_HID, HID], BF16, tag="wo")
        # normalization state lives past phase C (batch 1's normalization
        # overlaps batch 0's output projection)
        normp = ctx.enter_context(tc.tile_pool(name="normp", bufs=8))
        sums = [normp.tile([NH, T], FP32, tag=f"sums{b}", name=f"sums{b}")
                for b in range(BL)]
        psR = ctx.enter_context(
            tc.tile_pool(name="psR", bufs=2, space=bass.MemorySpace.PSUM))

        with ExitStack() as phase2:
            ptpool = phase2.enter_context(tc.tile_pool(name="ptpool", bufs=2))
            # PSUM budget is exactly 8 banks: ST pair tiles 2x2 (per-j
            # tiles, double-buffered, so ST(j+1) never waits for exp(j) to
            # drain an overlapping region) + o_ps 1 + cs 1 (the cs/OT/copy
            # interleave within a pair covers the single-buffer WARs) +
            # rank-1 broadcast 2 (outer pool).
            psS = phase2.enter_context(
                tc.tile_pool(name="psS", bufs=2, space=bass.MemorySpace.PSUM))
            psO = phase2.enter_context(
                tc.tile_pool(name="psO", bufs=1, space=bass.MemorySpace.PSUM))
            psC = phase2.enter_context(
                tc.tile_pool(name="psC", bufs=1, space=bass.MemorySpace.PSUM))

            mask_b = maskt_sb[:, None, :].to_broadcast((P, GROUPS, P))

            def emit_st(b, g):
                """ST matmuls + exp + mask for one head pair; returns pt."""
                pt = ptpool.tile([P, GROUPS, NTQ, T], BF16, tag="pt")
                for j in range(NTQ):
                    lo = j * P
                    st = psS.tile([P, GROUPS, T], FP32, tag="sps")
                    for hh in range(GROUPS):
                        h = GROUPS * g + hh
                        nc.tensor.matmul(
                            st[:, hh, lo:T],
                            kts[b][:, g, lo: lo + P],
                            qts[b][:, h, lo:T],
                            start=True, stop=True)
                    # one exp per (pair, j): 3D AP spanning both psum banks
                    # (no row-max: logits are O(1) by construction)
                    nc.scalar.activation(
                        out=pt[:, :, j, lo:T], in_=st[:, :, lo:T],
                        func=mybir.ActivationFunctionType.Exp,
                        bias=0.0, scale=1.0)
                    # causal mask on the diagonal block, both heads in one
                    # op (mask operand broadcast along the head axis), on
                    # the otherwise-idle gpsimd
                    nc.gpsimd.tensor_mul(
                        pt[:, :, j, lo:lo + P], pt[:, :, j, lo:lo + P],
                        mask_b)
                return pt

            def emit_csot_mm(b, g, pt):
                """colsum + OT accumulation; denominator stashed for the
                batched normalization."""
                for hh in range(GROUPS):
                    h = GROUPS * g + hh
                    o_ps = psO.tile([P, T], FP32, tag="ops")
                    cs_ps = psC.tile([1, T], FP32, tag="cps")
                    # colsum matmuls first so the denominator stash launches
                    # before the OT matmuls run
                    for j in range(NTQ):
                        lo = j * P
                        nc.tensor.matmul(
                            cs_ps[:, lo:T] if j else cs_ps[:, :],
                            ones_bf[:, 0:1],
                            pt[:, hh, j, lo:T],
                            start=(j == 0), stop=(j == NTQ - 1),
                            skip_group_check=True)
                    # copy the [1,T] row out of PSUM (split across ACT/DVE)
                    # and DMA-stash it into partition h of sums[b] so the
                    # whole batch's reciprocal runs 8-lane-wide later
                    # h0's copy on DVE (first DVE op of the pair, runs
                    # early), h1's on ACT: the next pair's first cs matmul
                    # WARs on h1's copy through the single cs bank, and ACT
                    # reaches it sooner than the backlogged DVE
                    csrow = normp.tile([1, T], FP32, tag="csrow")
                    if hh == 0:
                        nc.vector.tensor_copy(csrow, cs_ps)
                    else:
                        nc.scalar.copy(csrow, cs_ps)
                    nc.sync.dma_start(out=sums[b][h:h + 1, :], in_=csrow)
                    for j in range(NTQ):
                        lo = j * P
                        nc.tensor.matmul(
                            o_ps[:, lo:T] if j else o_ps[:, :],
                            v_sb[:, b * NTQ + j, g * D:(g + 1) * D],
                            pt[:, hh, j, lo:T],
                            start=(j == 0), stop=(j == NTQ - 1),
                            skip_group_check=True)
                    # unnormalized OT out of PSUM immediately (frees banks;
                    # the normalization multiply lands later, in-place)
                    nc.vector.tensor_copy(
                        ot_sb[:, h, b * T:(b + 1) * T], o_ps)

            def emit_batch_recip(b):
                """one 8-lane-wide reciprocal for the whole batch, then
                extract each head's row back to partition 0 for the rank-1
                broadcast. Runs while the next batch (or the output
                projection) keeps the PE busy."""
                rinv_f = normp.tile([NH, T], FP32, tag="rinvf",
                                    name=f"rinvf{b}")
                nc.vector.reciprocal_approx_fast(out=rinv_f, in_=sums[b])
                rinv_bf = normp.tile([NH, T], BF16, tag="rinvbf",
                                     name=f"rinvbf{b}")
                nc.vector.tensor_copy(rinv_bf, rinv_f)
                rrows = []
                for h in range(NH):
                    rrow = normp.tile([1, T], BF16, tag="rrow",
                                      name=f"rrow{b}_{h}")
                    nc.sync.dma_start(out=rrow, in_=rinv_bf[h:h + 1, :])
                    rrows.append(rrow)
                return rrows

            def emit_norm_heads(b, rrows, heads):
                """rank-1 broadcast + in-place multiply for a few heads;
                chunks are interspersed between matmul blocks so the PE
                never waits on the (cheap but latent) reciprocal chain."""
                for h in heads:
                    rb_ps = psR.tile([P, T], FP32, tag="rbps")
                    nc.tensor.matmul(rb_ps, ones_bf[0:1, :], rrows[h],
                                     start=True, stop=True,
                                     skip_group_check=True)
                    nc.vector.tensor_mul(
                        ot_sb[:, h, b * T:(b + 1) * T],
                        ot_sb[:, h, b * T:(b + 1) * T], rb_ps)

            # software-pipelined schedule: csot lags ST by one pair; batch
            # 0's normalization hides under batch 1's attention; batch 1's
            # hides under batch 0's output projection (emitted in phase D)
            pts = {}
            pairs = [(b, g) for b in range(BL) for g in range(NKV)]
            for b, g in pairs:
                pts[(b, g)] = None
            # wo's 2MB load trigger goes on the sync ring HERE: behind the
            # phase-1 loads but ahead of the norm stashes, landing long
            # before phase D needs it without blocking anything hot
            nc.sync.dma_start(out=wo_sb,
                              in_=wo.rearrange("(k p) n -> p k n", p=P))
            pts[(0, 0)] = emit_st(0, 0)
            for i in range(1, 4):
                pts[(0, i)] = emit_st(0, i)
                emit_csot_mm(0, i - 1, pts[(0, i - 1)])
            pts[(1, 0)] = emit_st(1, 0)
            emit_csot_mm(0, 3, pts[(0, 3)])
            pts[(1, 1)] = emit_st(1, 1)
            emit_csot_mm(1, 0, pts[(1, 0)])
            rrows0 = emit_batch_recip(0)
            emit_norm_heads(0, rrows0, [0, 1])
            pts[(1, 2)] = emit_st(1, 2)
            emit_csot_mm(1, 1, pts[(1, 1)])
            emit_norm_heads(0, rrows0, [2, 3])
            pts[(1, 3)] = emit_st(1, 3)
            emit_csot_mm(1, 2, pts[(1, 2)])
            emit_norm_heads(0, rrows0, [4, 5])
            emit_csot_mm(1, 3, pts[(1, 3)])
            emit_norm_heads(0, rrows0, [6, 7])
            rrows1 = emit_batch_recip(1)

        # ---- phase D: output projection (b0 tiles overlap b1's norm) ----
        with ExitStack() as phase3:
            opool = phase3.enter_context(tc.tile_pool(name="opool", bufs=3))
            psD = phase3.enter_context(
                tc.tile_pool(name="psD", bufs=3, space=bass.MemorySpace.PSUM))
            NCH = HID // T  # 2 chunks of 512

            def emit_out_tile(tt):
                o_tile = opool.tile([P, HID], BF16, tag="o")
                # interleave both output chunks k-major: consecutive matmul
                # pairs share the stationary operand OT[:,k,tt-block]
                ps0 = psD.tile([P, T], FP32, tag="dps0")
                ps1 = psD.tile([P, T], FP32, tag="dps1")
                pss = [ps0, ps1]
                for k in range(KT_HID):
                    for cchunk in range(NCH):
                        nc.tensor.matmul(
                            pss[cchunk],
                            ot_sb[:, k, tt * P:(tt + 1) * P],
                            wo_sb[:, k, cchunk * T:(cchunk + 1) * T],
                            start=(k == 0), stop=(k == KT_HID - 1))
                # alternate engines so the copies run in parallel; the last
                # tile is pure tail, so chunk it across engines and rings
                if tt < NTOK_T - 1:
                    nc.vector.tensor_copy(o_tile[:, 0:T], pss[0])
                    nc.scalar.copy(o_tile[:, T:HID], pss[1])
                    eng = nc.sync if tt % 2 == 0 else nc.scalar
                    eng.dma_start(out=out[tt * P:(tt + 1) * P, :], in_=o_tile)
                else:
                    HT = T // 2
                    nc.vector.tensor_copy(o_tile[:, 0:HT], pss[0][:, 0:HT])
                    nc.scalar.copy(o_tile[:, HT:T], pss[0][:, HT:T])
                    nc.vector.tensor_copy(o_tile[:, T:T + HT],
                                          pss[1][:, 0:HT])
                    nc.scalar.copy(o_tile[:, T + HT:HID], pss[1][:, HT:T])
                    nc.sync.dma_start(out=out[tt * P:(tt + 1) * P, 0:T],
                                      in_=o_tile[:, 0:T])
                    nc.scalar.dma_start(out=out[tt * P:(tt + 1) * P, T:HID],
                                        in_=o_tile[:, T:HID])

            emit_out_tile(0)
            emit_norm_heads(1, rrows1, [0, 1])
            emit_out_tile(1)
            emit_norm_heads(1, rrows1, [2, 3])
            emit_out_tile(2)
            emit_norm_heads(1, rrows1, [4, 5])
            emit_out_tile(3)
            emit_norm_heads(1, rrows1, [6, 7])
            for tt in range(4, NTOK_T):
                emit_out_tile(tt)


_COMPILED = None


def _get_compiled():
    global _COMPILED
    if _COMPILED is None:
        nc = bacc.Bacc("TRN2", target_bir_lowering=False, debug=False)
        _build(nc)
        nc.compile()
        _COMPILED = nc
    return _COMPILED


def kernel(hidden_states, Wq, Wk, Wv, Wo, _trace=False, _trace_kwargs=None):
    hs = np.asarray(hidden_states, dtype=np.float32).astype(BF)
    wq = np.ascontiguousarray(np.asarray(Wq, dtype=np.float32).astype(BF))
    wk = np.ascontiguousarray(np.asarray(Wk, dtype=np.float32).astype(BF))
    wv = np.ascontiguousarray(np.asarray(Wv, dtype=np.float32).astype(BF))
    wo = np.ascontiguousarray(np.asarray(Wo, dtype=np.float32).astype(BF))
    consts = _host_consts()
    nc = _get_compiled()
    in_maps = []
    for c in range(NCORES):
        # ship X pre-transposed ([HID, TOK]) so the kernel's lhs/rhs layouts
        # need no on-chip transpose of X at all
        shard_t = np.ascontiguousarray(
            hs[BL * c: BL * (c + 1)].reshape(TOK, HID).T)
        in_maps.append({"hidden_t": shard_t, "Wq": wq, "Wk": wk, "Wv": wv,
                        "Wo": wo, **consts})
    res = run_bass_kernel_spmd(
        nc, in_maps, list(range(NCORES)), trace=_trace,
        **(_trace_kwargs or {}))
    outs = [r["out"].astype(np.float32).reshape(BL, T, HID)
            for r in res.results]
    full = np.concatenate(outs, axis=0)
    if _trace:
        return full, res
    return full
